# revision 9
# baseline (speedup 1.0000x reference)
"""BNNLinear sampling kernel for Trainium2, data-parallel over 8 NeuronCores.

Computes h[m,c] = sum_r x_ext[m,r] * (mu[c,r] + sqrt(var[c,r]) * E[m,c,r])
with x_ext = concat([x, ones], axis=1), for
  x  [256, 512] f32, mu/var [512, 513] f32, E [256, 512, 513] f32.

Strategy (memory-bound; E is ~269 MB and must stream through HBM once):
 - Shard the sample axis m across the 8 cores (32 samples each).
 - Host-side LAYOUT ONLY: per-sample transpose of E to [r, c] blocked as
   [m, p, k, c] (r = 128k + p) so each per-sample DMA is one contiguous 1 MB
   transfer landing as SBUF tile [128p, 4k, 512c]; mu/var/x are pre-transposed
   the same way (tiny). All arithmetic (sqrt, multiplies, reductions) is
   on-chip.
 - Per sample: one DVE tensor_tensor B = E_t * sqrt(var)_t ([128, 2048]),
   then 4 fp32 PE matmuls (stationary = x column chunk [128, 1]) accumulate
   sum_r over the 4 r-chunks into a PSUM row -> h2[m, :].
 - mu term: one [M=32] PE matmul set (x_t @ mu_t) + bias row; the r=512 bias
   column of E contributes sqrt(var)[c,512] * E[m,c,512], computed as one
   tensor_tensor against a PE-broadcast sqrt(var) bias row.
 - PSUM rows are drained by DVE adds (h2 + (h1 + bias)) into an SBUF block
   laid out [32*(m%4) partition, m//4 block]; the final DMA scatters it to
   the DRAM output shard.
"""

import numpy as np
from contextlib import ExitStack

import concourse.bacc as bacc
import concourse.mybir as mybir
import concourse.tile as tile
from concourse.bass_utils import run_bass_kernel_spmd

F32 = mybir.dt.float32

N_CORES = 8
M_TOTAL = 256
M_SH = M_TOTAL // N_CORES  # 32 samples per core
C = 512
R_IN = 512                 # r chunks: 4 x 128
KCH = 4

_COMPILED = None


def _build_program(repeat=1):
    nc = bacc.Bacc("TRN2", target_bir_lowering=False, debug=False)

    et_d = nc.dram_tensor("et", [M_SH, 128, KCH, C], F32, kind="ExternalInput").ap()
    eb_d = nc.dram_tensor("eb", [M_SH, C], F32, kind="ExternalInput").ap()
    xt_d = nc.dram_tensor("xt", [128, KCH, M_SH], F32, kind="ExternalInput").ap()
    mu_d = nc.dram_tensor("mu_t", [128, KCH, C], F32, kind="ExternalInput").ap()
    mub_d = nc.dram_tensor("mu_b", [1, C], F32, kind="ExternalInput").ap()
    var_d = nc.dram_tensor("var_t", [128, KCH, C], F32, kind="ExternalInput").ap()
    varb_d = nc.dram_tensor("var_b", [1, C], F32, kind="ExternalInput").ap()
    out_d = nc.dram_tensor("out", [M_SH, C], F32, kind="ExternalOutput").ap()

    with tile.TileContext(nc) as tc, ExitStack() as ctx:
        const = ctx.enter_context(tc.tile_pool(name="const", bufs=1))
        work = ctx.enter_context(tc.tile_pool(name="work", bufs=8))
        bpool = ctx.enter_context(tc.tile_pool(name="bpool", bufs=5))
        opool = ctx.enter_context(tc.tile_pool(name="opool", bufs=1))
        psum = ctx.enter_context(tc.tile_pool(name="psum", bufs=4, space="PSUM"))
        psum1 = ctx.enter_context(tc.tile_pool(name="psum1", bufs=1, space="PSUM"))

        # ---- setup: constants in SBUF ----
        # order: var first (sqrt gates every TT), then x (gates matmuls),
        # then prefetch the first E tiles, then the drain-time constants.
        var_sb = const.tile([128, KCH, C], F32)
        nc.sync.dma_start(var_sb[:], var_d)
        varb_sb = const.tile([1, C], F32)
        nc.sync.dma_start(varb_sb[:], varb_d)
        xt_sb = const.tile([128, KCH, M_SH], F32)
        nc.sync.dma_start(xt_sb[:], xt_d)

        n_pre = 5
        pre_tiles = []
        for m in range(n_pre):
            e_t = work.tile([128, KCH, C], F32, tag="et")
            nc.sync.dma_start(e_t[:], et_d[m])
            pre_tiles.append(e_t)

        mu_sb = const.tile([128, KCH, C], F32)
        nc.sync.dma_start(mu_sb[:], mu_d)
        mub_sb = const.tile([1, C], F32)
        nc.sync.dma_start(mub_sb[:], mub_d)
        eb_sb = const.tile([M_SH, C], F32)
        nc.sync.dma_start(eb_sb[:], eb_d)

        # s = sqrt(var) in the blocked transposed layout
        s_sb = const.tile([128, KCH, C], F32)
        nc.scalar.sqrt(s_sb[:], var_sb[:])
        sb_sb = const.tile([1, C], F32)
        nc.scalar.sqrt(sb_sb[:], varb_sb[:])

        ones32 = const.tile([1, M_SH], F32)
        nc.vector.memset(ones32[:], 1.0)

        # broadcast sqrt(var) bias row to 32 partitions via PE outer product
        ps_b = psum1.tile([M_SH, C], F32)
        nc.tensor.matmul(ps_b[:], lhsT=ones32[:], rhs=sb_sb[:], start=True, stop=True)
        sbb_sb = const.tile([M_SH, C], F32)
        nc.scalar.copy(sbb_sb[:], ps_b[:])

        # h1 = x_t @ mu_t + mu bias row  -> [32, 512] psum, rows = samples
        h1_ps = psum1.tile([M_SH, C], F32)
        for k in range(KCH):
            nc.tensor.matmul(
                h1_ps[:], lhsT=xt_sb[:, k, :], rhs=mu_sb[:, k, :],
                start=(k == 0), stop=False,
            )
        nc.tensor.matmul(h1_ps[:], lhsT=ones32[:], rhs=mub_sb[:], start=False, stop=True)

        # hbs[m, c] = h1[m, c] + Eb[m, c] * sqrt(var)[c, 512]
        ebs_sb = const.tile([M_SH, C], F32)
        nc.vector.tensor_tensor(
            out=ebs_sb[:], in0=eb_sb[:], in1=sbb_sb[:], op=mybir.AluOpType.mult
        )
        hbs_sb = const.tile([M_SH, C], F32)
        nc.vector.tensor_tensor(
            out=hbs_sb[:], in0=h1_ps[:], in1=ebs_sb[:], op=mybir.AluOpType.add
        )

        # re-layout hbs rows: sample m = 4b + g  ->  partition 32g, block b
        # (one DMA per block: strided-dst APs with >1 free dim mis-lower)
        n_blk = M_SH // 4  # 8
        hbs_blk = const.tile([128, n_blk, C], F32)
        for b in range(n_blk):
            nc.sync.dma_start(hbs_blk[0:128:32, b, :], hbs_sb[4 * b : 4 * b + 4, :])

        out_sb = opool.tile([128, n_blk, C], F32)

        # ---- main loop over samples ----
        for r_i, b in [(rr, bb) for rr in range(repeat) for bb in range(n_blk)]:
            ps = psum.tile([128, C], F32, tag="ps")
            for g in range(4):
                m = 4 * b + g
                if r_i == 0 and m < n_pre:
                    e_t = pre_tiles[m]
                else:
                    e_t = work.tile([128, KCH, C], F32, tag="et")
                    nc.sync.dma_start(e_t[:], et_d[m])
                bt = bpool.tile([128, KCH, C], F32, tag="bt")
                nc.vector.tensor_tensor(
                    out=bt[:], in0=e_t[:], in1=s_sb[:], op=mybir.AluOpType.mult
                )
                for k in range(KCH):
                    nc.tensor.matmul(
                        ps[32 * g : 32 * g + 1, :],
                        lhsT=xt_sb[:, k, m : m + 1],
                        rhs=bt[:, k, :],
                        start=(k == 0),
                        stop=(k == KCH - 1),
                        tile_position=(0, 32 * g),
                    )
            # drain bank b: rows {0,32,64,96} -> out_sb block b (+ hbs)
            nc.vector.tensor_tensor(
                out=out_sb[0:97, b, :], in0=ps[0:97, :], in1=hbs_blk[0:97, b, :],
                op=mybir.AluOpType.add,
            )

        nc.sync.dma_start(
            out_d.rearrange("(b g) c -> g b c", g=4),
            out_sb[0:128:32, :, :],
        )

    nc.compile()
    return nc


def _prep_inputs(x, mu, var, E):
    x = np.ascontiguousarray(x, dtype=np.float32)
    mu = np.ascontiguousarray(mu, dtype=np.float32)
    var = np.ascontiguousarray(var, dtype=np.float32)
    E = np.ascontiguousarray(E, dtype=np.float32)

    # mu/var transposed-blocked: [p, k, c] with r = 128k + p (r < 512)
    def blk(t):
        tt = np.ascontiguousarray(t.T[:R_IN])          # [512, 512] (r, c)
        return np.ascontiguousarray(
            tt.reshape(KCH, 128, C).transpose(1, 0, 2)  # [128, 4, 512]
        )

    mu_t = blk(mu)
    var_t = blk(var)
    mu_b = np.ascontiguousarray(mu[:, R_IN]).reshape(1, C)
    var_b = np.ascontiguousarray(var[:, R_IN]).reshape(1, C)

    # E per-sample transpose + block: [m, p, k, c], r = 128k + p
    et = np.ascontiguousarray(
        E.transpose(0, 2, 1)[:, :R_IN, :]              # [256, 512(r), 512(c)]
        .reshape(M_TOTAL, KCH, 128, C)
        .transpose(0, 2, 1, 3)                          # [256, 128, 4, 512]
    )
    eb = np.ascontiguousarray(E[:, :, R_IN])            # [256, 512]

    # x transposed-blocked per core: [p, k, m_local]
    in_maps = []
    for core in range(N_CORES):
        sl = slice(core * M_SH, (core + 1) * M_SH)
        xs = x[sl]                                      # [32, 512]
        xt = np.ascontiguousarray(
            xs.T.reshape(KCH, 128, M_SH).transpose(1, 0, 2)  # [128, 4, 32]
        )
        in_maps.append({
            "et": np.ascontiguousarray(et[sl]),
            "eb": np.ascontiguousarray(eb[sl]),
            "xt": xt,
            "mu_t": mu_t,
            "var_t": var_t,
            "mu_b": mu_b,
            "var_b": var_b,
        })
    return in_maps


def kernel(x, mu, var, E, shape=None, _trace=False, **_ignored):
    global _COMPILED
    if _COMPILED is None:
        _COMPILED = _build_program()
    nc = _COMPILED
    in_maps = _prep_inputs(np.asarray(x), np.asarray(mu), np.asarray(var), np.asarray(E))
    res = run_bass_kernel_spmd(
        nc, in_maps, core_ids=list(range(N_CORES)), trace=_trace,
    )
    out = np.concatenate([res.results[i]["out"] for i in range(N_CORES)], axis=0)
    if _trace:
        kernel._last_results = res
    return out


# revision 14
# speedup vs baseline: 1.0309x; 1.0309x over previous
"""BNNLinear sampling kernel for Trainium2, data-parallel over 8 NeuronCores.

Computes h[m,c] = sum_r x_ext[m,r] * (mu[c,r] + sqrt(var[c,r]) * E[m,c,r])
with x_ext = concat([x, ones], axis=1), for
  x  [256, 512] f32, mu/var [512, 513] f32, E [256, 512, 513] f32.

Strategy (memory-bound; E is ~269 MB and must stream through HBM once):
 - Shard the sample axis m across the 8 cores (32 samples each).
 - Host-side LAYOUT ONLY: per-sample transpose of E to [r, c] blocked as
   [m, p, k, c] (r = 128k + p) so each per-sample DMA is one contiguous 1 MB
   transfer landing as SBUF tile [128p, 4k, 512c]; mu/var/x are pre-transposed
   the same way (tiny). All arithmetic (sqrt, multiplies, reductions) is
   on-chip.
 - Per sample: one DVE tensor_tensor B = E_t * sqrt(var)_t ([128, 2048]),
   then 4 fp32 PE matmuls (stationary = x column chunk [128, 1]) accumulate
   sum_r over the 4 r-chunks into a PSUM row -> h2[m, :].
 - mu term: one [M=32] PE matmul set (x_t @ mu_t) + bias row; the r=512 bias
   column of E contributes sqrt(var)[c,512] * E[m,c,512], computed as one
   tensor_tensor against a PE-broadcast sqrt(var) bias row.
 - PSUM rows are drained by DVE adds (h2 + (h1 + bias)) into an SBUF block
   laid out [32*(m%4) partition, m//4 block]; the final DMA scatters it to
   the DRAM output shard.
"""

import numpy as np
from contextlib import ExitStack

import concourse.bacc as bacc
import concourse.mybir as mybir
import concourse.tile as tile
from concourse.bass_utils import run_bass_kernel_spmd

F32 = mybir.dt.float32

N_CORES = 8
M_TOTAL = 256
M_SH = M_TOTAL // N_CORES  # 32 samples per core
C = 512
R_IN = 512                 # r chunks: 4 x 128
KCH = 4

_COMPILED = None


def _build_program(repeat=1):
    nc = bacc.Bacc("TRN2", target_bir_lowering=False, debug=False)

    et_d = nc.dram_tensor("et", [M_SH, 128, KCH, C], F32, kind="ExternalInput").ap()
    eb_d = nc.dram_tensor("eb", [M_SH, C], F32, kind="ExternalInput").ap()
    xt_d = nc.dram_tensor("xt", [128, KCH, M_SH], F32, kind="ExternalInput").ap()
    mu_d = nc.dram_tensor("mu_t", [128, KCH, C], F32, kind="ExternalInput").ap()
    mub_d = nc.dram_tensor("mu_b", [1, C], F32, kind="ExternalInput").ap()
    var_d = nc.dram_tensor("var_t", [128, KCH, C], F32, kind="ExternalInput").ap()
    varb_d = nc.dram_tensor("var_b", [1, C], F32, kind="ExternalInput").ap()
    out_d = nc.dram_tensor("out", [M_SH, C], F32, kind="ExternalOutput").ap()

    with tile.TileContext(nc) as tc, ExitStack() as ctx:
        const = ctx.enter_context(tc.tile_pool(name="const", bufs=1))
        work = ctx.enter_context(tc.tile_pool(name="work", bufs=6))
        bpool = ctx.enter_context(tc.tile_pool(name="bpool", bufs=4))
        opool = ctx.enter_context(tc.tile_pool(name="opool", bufs=1))
        psum = ctx.enter_context(tc.tile_pool(name="psum", bufs=4, space="PSUM"))
        psum1 = ctx.enter_context(tc.tile_pool(name="psum1", bufs=1, space="PSUM"))

        # ---- setup: constants in SBUF ----
        # The startup critical path is var -> sqrt -> first TT -> first MM.
        # Tile tracks deps per tile, so chunk the path with private tiles:
        # var arrives as 4 separate chunk tiles, each sqrt'd into its own
        # s-chunk tile; the first N_EARLY samples use per-chunk E DMAs/TTs
        # against those, so compute starts after ~1/4 of the var load.
        N_EARLY = 2
        xt_sb = const.tile([128, KCH, M_SH], F32)
        nc.sync.dma_start(xt_sb[:], xt_d)
        var_k = []
        s_k = []
        for k in range(KCH):
            v = const.tile([128, C], F32, tag=f"var{k}")
            nc.sync.dma_start(v[:], var_d[:, k, :])
            var_k.append(v)
            s = const.tile([128, C], F32, tag=f"s{k}")
            nc.scalar.sqrt(s[:], v[:])
            s_k.append(s)
        varb_sb = const.tile([1, C], F32)
        nc.sync.dma_start(varb_sb[:], varb_d)

        # early samples: fully chunked private pipelines
        early = []  # [m][k] -> bt chunk tile
        for m in range(N_EARLY):
            bts = []
            for k in range(KCH):
                e_c = const.tile([128, C], F32, tag=f"e_early{m}_{k}")
                nc.sync.dma_start(e_c[:], et_d[m, :, k, :])
                b_c = const.tile([128, C], F32, tag=f"b_early{m}_{k}")
                nc.vector.tensor_tensor(
                    out=b_c[:], in0=e_c[:], in1=s_k[k][:], op=mybir.AluOpType.mult
                )
                bts.append(b_c)
            early.append(bts)

        # steady-state s in one blocked tile (consumers wait for all chunks,
        # which is fine past the ramp)
        s_sb = const.tile([128, KCH, C], F32)
        for k in range(KCH):
            nc.scalar.copy(s_sb[:, k, :], s_k[k][:])
        sb_sb = const.tile([1, C], F32)
        nc.scalar.sqrt(sb_sb[:], varb_sb[:])

        n_pre = 5
        pre_tiles = [None] * N_EARLY
        for m in range(N_EARLY, n_pre):
            e_t = work.tile([128, KCH, C], F32, tag="et")
            nc.sync.dma_start(e_t[:], et_d[m])
            pre_tiles.append(e_t)

        mu_sb = const.tile([128, KCH, C], F32)
        nc.sync.dma_start(mu_sb[:], mu_d)
        mub_sb = const.tile([1, C], F32)
        nc.sync.dma_start(mub_sb[:], mub_d)
        eb_sb = const.tile([M_SH, C], F32)
        nc.sync.dma_start(eb_sb[:], eb_d)

        ones32 = const.tile([1, M_SH], F32)
        nc.vector.memset(ones32[:], 1.0)

        # broadcast sqrt(var) bias row to 32 partitions via PE outer product
        ps_b = psum1.tile([M_SH, C], F32)
        nc.tensor.matmul(ps_b[:], lhsT=ones32[:], rhs=sb_sb[:], start=True, stop=True)
        sbb_sb = const.tile([M_SH, C], F32)
        nc.scalar.copy(sbb_sb[:], ps_b[:])

        # h1 = x_t @ mu_t + mu bias row  -> [32, 512] psum, rows = samples
        h1_ps = psum1.tile([M_SH, C], F32)
        for k in range(KCH):
            nc.tensor.matmul(
                h1_ps[:], lhsT=xt_sb[:, k, :], rhs=mu_sb[:, k, :],
                start=(k == 0), stop=False,
            )
        nc.tensor.matmul(h1_ps[:], lhsT=ones32[:], rhs=mub_sb[:], start=False, stop=True)

        # hbs[m, c] = h1[m, c] + Eb[m, c] * sqrt(var)[c, 512]
        ebs_sb = const.tile([M_SH, C], F32)
        nc.vector.tensor_tensor(
            out=ebs_sb[:], in0=eb_sb[:], in1=sbb_sb[:], op=mybir.AluOpType.mult
        )
        hbs_sb = const.tile([M_SH, C], F32)
        nc.vector.tensor_tensor(
            out=hbs_sb[:], in0=h1_ps[:], in1=ebs_sb[:], op=mybir.AluOpType.add
        )

        # re-layout hbs rows: sample m = 4b + g  ->  partition 32g, block b
        # (one DMA per block: strided-dst APs with >1 free dim mis-lower)
        n_blk = M_SH // 4  # 8
        hbs_blk = const.tile([128, n_blk, C], F32)
        for b in range(n_blk):
            nc.sync.dma_start(hbs_blk[0:128:32, b, :], hbs_sb[4 * b : 4 * b + 4, :])

        out_sb = opool.tile([128, n_blk, C], F32)

        # ---- main loop over samples ----
        for r_i, b in [(rr, bb) for rr in range(repeat) for bb in range(n_blk)]:
            ps = psum.tile([128, C], F32, tag="ps")
            for g in range(4):
                m = 4 * b + g
                if r_i == 0 and m < N_EARLY:
                    bt_chunks = early[m]
                else:
                    if r_i == 0 and m < n_pre:
                        e_t = pre_tiles[m]
                    else:
                        e_t = work.tile([128, KCH, C], F32, tag="et")
                        nc.sync.dma_start(e_t[:], et_d[m])
                    bt = bpool.tile([128, KCH, C], F32, tag="bt")
                    nc.vector.tensor_tensor(
                        out=bt[:], in0=e_t[:], in1=s_sb[:], op=mybir.AluOpType.mult
                    )
                    bt_chunks = [bt[:, k, :] for k in range(KCH)]
                for k in range(KCH):
                    nc.tensor.matmul(
                        ps[32 * g : 32 * g + 1, :],
                        lhsT=xt_sb[:, k, m : m + 1],
                        rhs=bt_chunks[k],
                        start=(k == 0),
                        stop=(k == KCH - 1),
                        tile_position=(0, 32 * g),
                    )
            # drain bank b: rows {0,32,64,96} -> out_sb block b (+ hbs)
            nc.vector.tensor_tensor(
                out=out_sb[0:97, b, :], in0=ps[0:97, :], in1=hbs_blk[0:97, b, :],
                op=mybir.AluOpType.add,
            )

        nc.sync.dma_start(
            out_d.rearrange("(b g) c -> g b c", g=4),
            out_sb[0:128:32, :, :],
        )

    nc.compile()
    return nc


def _prep_inputs(x, mu, var, E):
    x = np.ascontiguousarray(x, dtype=np.float32)
    mu = np.ascontiguousarray(mu, dtype=np.float32)
    var = np.ascontiguousarray(var, dtype=np.float32)
    E = np.ascontiguousarray(E, dtype=np.float32)

    # mu/var transposed-blocked: [p, k, c] with r = 128k + p (r < 512)
    def blk(t):
        tt = np.ascontiguousarray(t.T[:R_IN])          # [512, 512] (r, c)
        return np.ascontiguousarray(
            tt.reshape(KCH, 128, C).transpose(1, 0, 2)  # [128, 4, 512]
        )

    mu_t = blk(mu)
    var_t = blk(var)
    mu_b = np.ascontiguousarray(mu[:, R_IN]).reshape(1, C)
    var_b = np.ascontiguousarray(var[:, R_IN]).reshape(1, C)

    # E per-sample transpose + block: [m, p, k, c], r = 128k + p
    et = np.ascontiguousarray(
        E.transpose(0, 2, 1)[:, :R_IN, :]              # [256, 512(r), 512(c)]
        .reshape(M_TOTAL, KCH, 128, C)
        .transpose(0, 2, 1, 3)                          # [256, 128, 4, 512]
    )
    eb = np.ascontiguousarray(E[:, :, R_IN])            # [256, 512]

    # x transposed-blocked per core: [p, k, m_local]
    in_maps = []
    for core in range(N_CORES):
        sl = slice(core * M_SH, (core + 1) * M_SH)
        xs = x[sl]                                      # [32, 512]
        xt = np.ascontiguousarray(
            xs.T.reshape(KCH, 128, M_SH).transpose(1, 0, 2)  # [128, 4, 32]
        )
        in_maps.append({
            "et": np.ascontiguousarray(et[sl]),
            "eb": np.ascontiguousarray(eb[sl]),
            "xt": xt,
            "mu_t": mu_t,
            "var_t": var_t,
            "mu_b": mu_b,
            "var_b": var_b,
        })
    return in_maps


def kernel(x, mu, var, E, shape=None, _trace=False, **_ignored):
    global _COMPILED
    if _COMPILED is None:
        _COMPILED = _build_program()
    nc = _COMPILED
    in_maps = _prep_inputs(np.asarray(x), np.asarray(mu), np.asarray(var), np.asarray(E))
    res = run_bass_kernel_spmd(
        nc, in_maps, core_ids=list(range(N_CORES)), trace=_trace,
    )
    out = np.concatenate([res.results[i]["out"] for i in range(N_CORES)], axis=0)
    if _trace:
        kernel._last_results = res
    return out
